# revision 22
# baseline (speedup 1.0000x reference)
"""Duration-based length regulation (KittenTTS LengthRegulator) on 8 trn2 NeuronCores.

For each batch b (one per core): phoneme t's feature row is repeated
clamp(durations[b,t],1) times along the frame axis; frames are zero-padded to
MAX_LEN = T*15 (pad rows are never written: the runners hand the kernel
pre-zeroed output buffers).

Raw-bass kernel (no TileContext): Tile's auto-dependency tracking daisy-chains
consecutive SWDGE scatters on the shared output tensor (each waits for the
previous one to fully drain), which serialized the baseline to ~6x the HBM
roofline. Five engine streams with manual semaphores instead. DMA completion
semaphores arrive ~2-3us after the transfer, and every cross-engine handoff
costs 1-2us, so the schedule minimizes hops on the path to the first scatter:

  SP ring:    durations (int + f32 views), mask constants, feature tiles 0-1.
  ACT ring:   the matmul constant first (gates the PE), feature tiles 2-3;
              the ACT engine then builds replicas for tiles 2-3.
  PE:         exclusive global cumsum straight from the loaded f32 durations:
              strict-upper-tri matmul (within-column partial sums) + all-ones
              matmul (column sums). No DVE dependency.
  DVE:        per-pass additive terms hi_s = (dur & -(2s)) + OOB*((dur&s)==0)
              for s in {1,2,4,8} (binary block decomposition; OOB pushes masked
              descriptors past bounds_check so the ucode skips them), replica
              levels for tiles 0-1 interleaved with the offset chain
              offs_s = (partial + shifted column sums) + hi_s.
              Every DVE/ACT op bumps a counter sem and dependent ops wait on it
              (the pipelined engines do not order same-engine RAW).
  Pool:       a 4-byte SWDGE warm-up DMA (the first SWDGE op pays ~1us of
              ucode warm-up), then 16 indirect scatter DMAs ([128,1] offsets -
              the only offset shape the HW ucode supports) issued back-to-back
              in pass order s=4,8,2,1 so the 16 SDMA engines never starve.
              One final wait for all 256 completion increments.

Each output row is written exactly once -> DMA write traffic == sum(dur) rows
(~8 MB/core), which is the HBM-write roofline for this kernel.
"""

import sys

import numpy as np

if "/opt/trn_rl_repo" not in sys.path:
    sys.path.insert(0, "/opt/trn_rl_repo")

B, T, D = 8, 512, 512
MAX_DUR = 15
MAX_LEN = T * MAX_DUR  # 7680
P = 128
NT = T // P  # 4 duration columns / feature tiles
SBLK = [1, 2, 4, 8]  # pass sizes in offset-column order (issue order differs)
ISSUE = [4, 8, 2, 1]  # scatter issue order: feed the SDMA engines big passes first
OOB = 1 << 20  # pushed past bounds_check -> descriptor silently skipped
SMAX = 8
DVE_TILES = (0, 1)  # replica doubling on DVE
ACT_TILES = (2, 3)  # replica doubling on ACT

_CACHE = {}


def _host_constants():
    """Input-independent constant tensors shipped with every batch."""
    # tri[:, 0:128]: strict upper triangular ones (lhsT for exclusive cumsum
    # along partitions: (tri.T @ x)[p] = sum_{p'<p} x[p']).
    # tri[:, 128:256]: all ones (column sums, broadcast to every partition).
    tri = np.zeros((P, 2 * P), dtype=np.float32)
    tri[:, :P] = np.triu(np.ones((P, P), dtype=np.float32), k=1)
    tri[:, P:] = 1.0
    # ic[:, 0:16]:  -(2s) per pass column group (hi = dur & -(2s))
    # ic[:, 16:32]: s bit per pass column group (mask = dur & s)
    ic = np.zeros((P, 32), dtype=np.int32)
    for si, s in enumerate(SBLK):
        ic[:, si * NT : (si + 1) * NT] = -(2 * s)
        ic[:, 16 + si * NT : 16 + (si + 1) * NT] = s
    return tri, ic


def _build_nc():
    from concourse import bass, mybir
    from concourse.bacc import Bacc

    f32, i32 = mybir.dt.float32, mybir.dt.int32
    Alu = mybir.AluOpType

    nc = Bacc(num_swdge_queues=2)
    feats = nc.declare_dram_parameter("features", [T, D], f32, isOutput=False)
    durs_mat = nc.declare_dram_parameter("durations_t", [P, NT], i32, isOutput=False)
    durs_f = nc.declare_dram_parameter("durations_f", [P, NT], f32, isOutput=False)
    tri_c = nc.declare_dram_parameter("tri_const", [P, 2 * P], f32, isOutput=False)
    int_c = nc.declare_dram_parameter("int_const", [P, 32], i32, isOutput=False)
    out = nc.declare_dram_parameter("out", [MAX_LEN, D], f32, isOutput=True)

    # SBUF: replication tile j occupies cols [j*SMAX*D, (j+1)*SMAX*D);
    # replica r of row (j*128+p) sits at rep[p, j*SMAX*D + r*D : .. + D]
    rep = nc.alloc_sbuf_tensor("rep", [P, NT * SMAX * D], f32)
    dur_sb = nc.alloc_sbuf_tensor("dur_sb", [P, NT], i32)
    durf = nc.alloc_sbuf_tensor("durf", [P, NT], f32)
    tri_sb = nc.alloc_sbuf_tensor("tri_sb", [P, 2 * P], f32)
    ic_sb = nc.alloc_sbuf_tensor("ic_sb", [P, 32], i32)
    dur4 = nc.alloc_sbuf_tensor("dur4", [P, 4 * NT], i32)
    excf = nc.alloc_sbuf_tensor("excf", [P, NT], f32)
    exci = nc.alloc_sbuf_tensor("exci", [P, NT], i32)
    offs = nc.alloc_sbuf_tensor("offs", [P, 4 * NT], i32)
    hi4 = nc.alloc_sbuf_tensor("hi4", [P, 4 * NT], i32)
    m4 = nc.alloc_sbuf_tensor("m4", [P, 4 * NT], i32)
    warm = nc.alloc_sbuf_tensor("warm", [2, 1], i32)
    warmd = nc.alloc_sbuf_tensor("warmd", [2, D], f32)
    ps = nc.alloc_psum_tensor("ps", [P, NT], f32)
    cs = nc.alloc_psum_tensor("cs", [P, NT], f32)

    s_dur = nc.alloc_semaphore("s_dur")  # int durations load
    s_df = nc.alloc_semaphore("s_df")  # f32 durations load
    s_tri = nc.alloc_semaphore("s_tri")  # matmul-constant load
    s_ic = nc.alloc_semaphore("s_ic")  # mask-constant load
    s_f = [nc.alloc_semaphore(f"s_f{j}") for j in range(NT)]  # feature loads
    s_mm = nc.alloc_semaphore("s_mm")  # PE cumsum done
    s_v = nc.alloc_semaphore("s_v")  # DVE op counter
    s_a = nc.alloc_semaphore("s_a")  # ACT copy counter
    s_w = nc.alloc_semaphore("s_w")  # SWDGE warm-up DMA
    s_p = nc.alloc_semaphore("s_p")  # Pool memset counter
    s_sc = nc.alloc_semaphore("s_sc")  # scatter DMA completions


    with nc.Block(no_gpsimd_drain=True) as blk:

        @blk.sync
        def _(sync):
            sync.dma_start(out=dur_sb[:], in_=durs_mat[:, :]).then_inc(s_dur, 16)
            sync.dma_start(out=ic_sb[:], in_=int_c[:, :]).then_inc(s_ic, 16)
            for j in DVE_TILES:
                sync.dma_start(
                    out=rep[:, j * SMAX * D : j * SMAX * D + D],
                    in_=feats[j * P : (j + 1) * P, :],
                ).then_inc(s_f[j], 16)

        @blk.scalar
        def _(scalar):
            scalar.dma_start(out=durf[:], in_=durs_f[:, :]).then_inc(s_df, 16)
            scalar.dma_start(out=tri_sb[:], in_=tri_c[:, :]).then_inc(s_tri, 16)
            for j in ACT_TILES:
                scalar.dma_start(
                    out=rep[:, j * SMAX * D : j * SMAX * D + D],
                    in_=feats[j * P : (j + 1) * P, :],
                ).then_inc(s_f[j], 16)

            # replicas for ACT_TILES: w1(t2), w1(t3), w2(t2), w2(t3), ...
            na = 0
            for w in (1, 2, 4):
                for j in ACT_TILES:
                    if w == 1:
                        scalar.wait_ge(s_f[j], 16)
                    else:
                        scalar.wait_ge(s_a, na - 1)  # same tile's previous level
                    base = j * SMAX * D
                    scalar.copy(
                        out=rep[:, base + w * D : base + 2 * w * D],
                        in_=rep[:, base : base + w * D],
                    ).then_inc(s_a, 1)
                    na += 1

        @blk.tensor
        def _(tensor):
            tensor.wait_ge(s_df, 16)
            tensor.wait_ge(s_tri, 16)
            tensor.matmul(ps[:, :], tri_sb[:, 0:P], durf[:, :], start=True, stop=True)
            tensor.matmul(
                cs[:, :], tri_sb[:, P : 2 * P], durf[:, :], start=True, stop=True
            ).then_inc(s_mm, 1)

        @blk.vector
        def _(vector):
            n = 0  # s_v value after each op below

            def op(inst):
                nonlocal n
                n += 1
                return inst.then_inc(s_v, 1)

            def dep(k):
                vector.wait_ge(s_v, k)

            def rep_copy(j, w):
                base = j * SMAX * D
                return vector.tensor_copy(
                    out=rep[:, base + w * D : base + 2 * w * D],
                    in_=rep[:, base : base + w * D],
                )

            # --- replicated int durations + per-pass hi terms (pre-matmul)
            vector.wait_ge(s_dur, 16)
            op(vector.tensor_copy(out=dur4[:, 0:NT], in_=dur_sb[:]))  # 1
            dep(1)
            op(vector.tensor_copy(out=dur4[:, NT : 2 * NT], in_=dur4[:, 0:NT]))  # 2
            dep(2)
            op(vector.tensor_copy(out=dur4[:, 2 * NT : 4 * NT], in_=dur4[:, 0 : 2 * NT]))  # 3
            vector.wait_ge(s_ic, 16)
            dep(3)
            op(vector.tensor_tensor(
                out=hi4[:], in0=dur4[:], in1=ic_sb[:, 0:16], op=Alu.bitwise_and
            ))  # 4
            dep(3)
            op(vector.tensor_tensor(
                out=m4[:], in0=dur4[:], in1=ic_sb[:, 16:32], op=Alu.bitwise_and
            ))  # 5
            dep(5)
            op(vector.tensor_scalar(
                out=m4[:], in0=m4[:], scalar1=0, scalar2=OOB,
                op0=Alu.is_equal, op1=Alu.mult,
            ))  # 6
            dep(6)
            op(vector.tensor_tensor(out=hi4[:], in0=hi4[:], in1=m4[:], op=Alu.add))  # 7

            # --- exc, then per-pass offset groups (s=4 group first)
            vector.wait_ge(s_mm, 1)
            op(vector.tensor_copy(out=excf[:], in_=ps[:, :]))  # 8
            for sh in range(1, NT):
                dep(n)
                op(vector.tensor_tensor(
                    out=excf[:, sh:NT], in0=excf[:, sh:NT],
                    in1=cs[:, 0 : NT - sh], op=Alu.add,
                ))  # 9,10,11
            dep(11)
            op(vector.tensor_copy(out=exci[:], in_=excf[:]))  # 12 (f32->i32)
            for si in (2, 3, 1, 0):  # pass groups s=4,8,2,1
                dep(12)
                op(vector.tensor_tensor(
                    out=offs[:, si * NT : (si + 1) * NT], in0=exci[:],
                    in1=hi4[:, si * NT : (si + 1) * NT], op=Alu.add,
                ))  # 13 (s4), 14 (s8), 15 (s2), 16 (s1)

            # --- replicas for DVE_TILES
            vector.wait_ge(s_f[0], 16)
            op(rep_copy(0, 1))  # 17
            vector.wait_ge(s_f[1], 16)
            op(rep_copy(1, 1))  # 18
            dep(17)
            op(rep_copy(0, 2))  # 19
            dep(18)
            op(rep_copy(1, 2))  # 20
            dep(19)
            op(rep_copy(0, 4))  # 21
            dep(20)
            op(rep_copy(1, 4))  # 22

        @blk.gpsimd
        def _(gpsimd):
            # SWDGE indirect-ucode warm-up: 2 descriptors, both pushed past
            # bounds_check=0 -> nothing is written, but the Q7 indirect kernel
            # is hot before the real scatters
            gpsimd.memset(warm[:], OOB).then_inc(s_p, 1)
            gpsimd.memset(warmd[:], 0.0).then_inc(s_p, 1)
            gpsimd.wait_ge(s_p, 2)
            gpsimd.indirect_dma_start(
                out=out[:, :],
                out_offset=bass.IndirectOffsetOnAxis(ap=warm[0:2, 0:1], axis=0),
                in_=warmd[0:2, 0:D],
                in_offset=None,
                bounds_check=nc.gpsimd.to_reg(0),
                oob_is_err=False,
            ).then_inc(s_w, 16)
            bregs = {s_: gpsimd.to_reg(MAX_LEN - s_) for s_ in SBLK}
            for j in range(NT):
                gpsimd.wait_ge(s_f[j], 16)
            # per-scatter gates: scatter (s, tile j) waits on its own offset
            # group + tile j's replica level; DVE tiles (0,1) finish first
            v_gate = {4: {0: 19, 1: 20}, 8: {0: 21, 1: 22}, 2: {}, 1: {}}
            a_gate = {4: {2: 3, 3: 4}, 8: {2: 5, 3: 6}, 2: {2: 1, 3: 2}, 1: {}}
            v_offs = {4: 13, 8: 14, 2: 15, 1: 16}
            for s_ in ISSUE:
                si = SBLK.index(s_)
                for j in (0, 1, 2, 3):
                    gpsimd.wait_ge(s_v, max(v_offs[s_], v_gate[s_].get(j, 0)))
                    if j in a_gate[s_]:
                        gpsimd.wait_ge(s_a, a_gate[s_][j])
                    inst = gpsimd.indirect_dma_start(
                        out=out[:, :],
                        out_offset=bass.IndirectOffsetOnAxis(
                            ap=offs[:, si * NT + j : si * NT + j + 1], axis=0
                        ),
                        in_=rep[:, j * SMAX * D : j * SMAX * D + s_ * D],
                        in_offset=None,
                        bounds_check=bregs[s_],
                        oob_is_err=False,
                    ).then_inc(s_sc, 16)
                    if j % 2:
                        inst.ins.queue = "qPoolDynamic1"
            gpsimd.wait_ge(s_sc, 16 * 4 * NT)  # all 16 scatters drained
            gpsimd.wait_ge(s_w, 16)

    nc.compile()
    return nc


def _get_nc():
    if "nc" not in _CACHE:
        _CACHE["nc"] = _build_nc()
    return _CACHE["nc"]


def _run(features, durations, trace=False):
    """features (B,T,D) f32, durations (B,T) i32 -> (out (B,MAX_LEN,D) f32, results)."""
    from concourse.bass_utils import run_bass_kernel_spmd

    nc = _get_nc()
    tri, ic = _host_constants()
    in_maps = []
    for b in range(B):
        durc = np.maximum(durations[b], 1)  # clamp(min=1), as in forward()
        dmat = np.ascontiguousarray(durc.reshape(NT, P).T)  # [P, NT]
        in_maps.append(
            {
                "features": np.ascontiguousarray(features[b]),
                "durations_t": dmat,
                "durations_f": dmat.astype(np.float32),
                "tri_const": tri,
                "int_const": ic,
            }
        )
    kwargs = {}
    if trace:
        kwargs = dict(trace=True, trace_cores=list(range(B)), stitch_traces=False)
    res = run_bass_kernel_spmd(nc, in_maps, core_ids=list(range(B)), **kwargs)
    outs = np.stack([res.results[b]["out"] for b in range(B)])
    return outs.astype(np.float32, copy=False), res


def kernel(features, durations):
    features = np.asarray(features, dtype=np.float32)
    durations = np.asarray(durations, dtype=np.int32)
    outs, _ = _run(features, durations, trace=False)
    return outs


if __name__ == "__main__":
    feats = np.random.randn(B, T, D).astype(np.float32)
    durs = np.random.randint(0, 16, size=(B, T)).astype(np.int32)
    out = kernel(feats, durs)
    print("out", out.shape, out.dtype)


# revision 23
# speedup vs baseline: 1.0158x; 1.0158x over previous
"""Duration-based length regulation (KittenTTS LengthRegulator) on 8 trn2 NeuronCores.

For each batch b (one per core): phoneme t's feature row is repeated
clamp(durations[b,t],1) times along the frame axis; frames are zero-padded to
MAX_LEN = T*15 (pad rows are never written: the runners hand the kernel
pre-zeroed output buffers).

Raw-bass kernel (no TileContext): Tile's auto-dependency tracking daisy-chains
consecutive SWDGE scatters on the shared output tensor (each waits for the
previous one to fully drain), which serialized the baseline to ~6x the HBM
roofline. Five engine streams with manual semaphores instead. DMA completion
semaphores arrive ~2-3us after the transfer, and every cross-engine handoff
costs 1-2us, so the schedule minimizes hops on the path to the first scatter:

  SP ring:    durations (int + f32 views), mask constants, feature tiles 0-1.
  ACT ring:   the matmul constant first (gates the PE), feature tiles 2-3;
              the ACT engine then builds replicas for tiles 2-3.
  PE:         exclusive global cumsum straight from the loaded f32 durations:
              strict-upper-tri matmul (within-column partial sums) + all-ones
              matmul (column sums). No DVE dependency.
  DVE:        per-pass additive terms hi_s = (dur & -(2s)) + OOB*((dur&s)==0)
              for s in {1,2,4,8} (binary block decomposition; OOB pushes masked
              descriptors past bounds_check so the ucode skips them), replica
              levels for tiles 0-1 interleaved with the offset chain
              offs_s = (partial + shifted column sums) + hi_s.
              Every DVE/ACT op bumps a counter sem and dependent ops wait on it
              (the pipelined engines do not order same-engine RAW).
  Pool:       a 4-byte SWDGE warm-up DMA (the first SWDGE op pays ~1us of
              ucode warm-up), then 16 indirect scatter DMAs ([128,1] offsets -
              the only offset shape the HW ucode supports) issued back-to-back
              in pass order s=4,8,2,1 so the 16 SDMA engines never starve.
              One final wait for all 256 completion increments.

Each output row is written exactly once -> DMA write traffic == sum(dur) rows
(~8 MB/core), which is the HBM-write roofline for this kernel.
"""

import sys

import numpy as np

if "/opt/trn_rl_repo" not in sys.path:
    sys.path.insert(0, "/opt/trn_rl_repo")

B, T, D = 8, 512, 512
MAX_DUR = 15
MAX_LEN = T * MAX_DUR  # 7680
P = 128
NT = T // P  # 4 duration columns / feature tiles
SBLK = [1, 2, 4, 8]  # pass sizes in offset-column order (issue order differs)
ISSUE = [4, 8, 2, 1]  # scatter issue order: feed the SDMA engines big passes first
OOB = 1 << 20  # pushed past bounds_check -> descriptor silently skipped
SMAX = 8
DVE_TILES = (0, 1)  # replica doubling on DVE
ACT_TILES = (2, 3)  # replica doubling on ACT

_CACHE = {}


def _host_constants():
    """Input-independent constant tensors shipped with every batch."""
    # tri[:, 0:128]: strict upper triangular ones (lhsT for exclusive cumsum
    # along partitions: (tri.T @ x)[p] = sum_{p'<p} x[p']).
    # tri[:, 128:256]: all ones (column sums, broadcast to every partition).
    tri = np.zeros((P, 2 * P), dtype=np.float32)
    tri[:, :P] = np.triu(np.ones((P, P), dtype=np.float32), k=1)
    tri[:, P:] = 1.0
    # ic[:, 0:16]:  -(2s) per pass column group (hi = dur & -(2s))
    # ic[:, 16:32]: s bit per pass column group (mask = dur & s)
    ic = np.zeros((P, 32), dtype=np.int32)
    for si, s in enumerate(SBLK):
        ic[:, si * NT : (si + 1) * NT] = -(2 * s)
        ic[:, 16 + si * NT : 16 + (si + 1) * NT] = s
    return tri, ic


def _build_nc():
    from concourse import bass, mybir
    from concourse.bacc import Bacc

    f32, i32 = mybir.dt.float32, mybir.dt.int32
    Alu = mybir.AluOpType

    nc = Bacc(num_swdge_queues=4)
    feats = nc.declare_dram_parameter("features", [T, D], f32, isOutput=False)
    durs_mat = nc.declare_dram_parameter("durations_t", [P, NT], i32, isOutput=False)
    durs_f = nc.declare_dram_parameter("durations_f", [P, NT], f32, isOutput=False)
    tri_c = nc.declare_dram_parameter("tri_const", [P, 2 * P], f32, isOutput=False)
    int_c = nc.declare_dram_parameter("int_const", [P, 32], i32, isOutput=False)
    out = nc.declare_dram_parameter("out", [MAX_LEN, D], f32, isOutput=True)

    # SBUF: replication tile j occupies cols [j*SMAX*D, (j+1)*SMAX*D);
    # replica r of row (j*128+p) sits at rep[p, j*SMAX*D + r*D : .. + D]
    rep = nc.alloc_sbuf_tensor("rep", [P, NT * SMAX * D], f32)
    dur_sb = nc.alloc_sbuf_tensor("dur_sb", [P, NT], i32)
    durf = nc.alloc_sbuf_tensor("durf", [P, NT], f32)
    tri_sb = nc.alloc_sbuf_tensor("tri_sb", [P, 2 * P], f32)
    ic_sb = nc.alloc_sbuf_tensor("ic_sb", [P, 32], i32)
    dur4 = nc.alloc_sbuf_tensor("dur4", [P, 4 * NT], i32)
    excf = nc.alloc_sbuf_tensor("excf", [P, NT], f32)
    exci = nc.alloc_sbuf_tensor("exci", [P, NT], i32)
    offs = nc.alloc_sbuf_tensor("offs", [P, 4 * NT], i32)
    hi4 = nc.alloc_sbuf_tensor("hi4", [P, 4 * NT], i32)
    m4 = nc.alloc_sbuf_tensor("m4", [P, 4 * NT], i32)
    warm = nc.alloc_sbuf_tensor("warm", [2, 1], i32)
    warmd = nc.alloc_sbuf_tensor("warmd", [2, D], f32)
    ps = nc.alloc_psum_tensor("ps", [P, NT], f32)
    cs = nc.alloc_psum_tensor("cs", [P, NT], f32)

    s_dur = nc.alloc_semaphore("s_dur")  # int durations load
    s_df = nc.alloc_semaphore("s_df")  # f32 durations load
    s_tri = nc.alloc_semaphore("s_tri")  # matmul-constant load
    s_ic = nc.alloc_semaphore("s_ic")  # mask-constant load
    s_f = [nc.alloc_semaphore(f"s_f{j}") for j in range(NT)]  # feature loads
    s_mm = nc.alloc_semaphore("s_mm")  # PE cumsum done
    s_v = nc.alloc_semaphore("s_v")  # DVE op counter
    s_a = nc.alloc_semaphore("s_a")  # ACT copy counter
    s_w = nc.alloc_semaphore("s_w")  # SWDGE warm-up DMA
    s_p = nc.alloc_semaphore("s_p")  # Pool memset counter
    s_sc = nc.alloc_semaphore("s_sc")  # scatter DMA completions


    with nc.Block(no_gpsimd_drain=True) as blk:

        @blk.sync
        def _(sync):
            sync.dma_start(out=dur_sb[:], in_=durs_mat[:, :]).then_inc(s_dur, 16)
            sync.dma_start(out=ic_sb[:], in_=int_c[:, :]).then_inc(s_ic, 16)
            for j in DVE_TILES:
                sync.dma_start(
                    out=rep[:, j * SMAX * D : j * SMAX * D + D],
                    in_=feats[j * P : (j + 1) * P, :],
                ).then_inc(s_f[j], 16)

        @blk.scalar
        def _(scalar):
            scalar.dma_start(out=durf[:], in_=durs_f[:, :]).then_inc(s_df, 16)
            scalar.dma_start(out=tri_sb[:], in_=tri_c[:, :]).then_inc(s_tri, 16)
            for j in ACT_TILES:
                scalar.dma_start(
                    out=rep[:, j * SMAX * D : j * SMAX * D + D],
                    in_=feats[j * P : (j + 1) * P, :],
                ).then_inc(s_f[j], 16)

            # replicas for ACT_TILES: w1(t2), w1(t3), w2(t2), w2(t3), ...
            na = 0
            for w in (1, 2, 4):
                for j in ACT_TILES:
                    if w == 1:
                        scalar.wait_ge(s_f[j], 16)
                    else:
                        scalar.wait_ge(s_a, na - 1)  # same tile's previous level
                    base = j * SMAX * D
                    scalar.copy(
                        out=rep[:, base + w * D : base + 2 * w * D],
                        in_=rep[:, base : base + w * D],
                    ).then_inc(s_a, 1)
                    na += 1

        @blk.tensor
        def _(tensor):
            tensor.wait_ge(s_df, 16)
            tensor.wait_ge(s_tri, 16)
            tensor.matmul(ps[:, :], tri_sb[:, 0:P], durf[:, :], start=True, stop=True)
            tensor.matmul(
                cs[:, :], tri_sb[:, P : 2 * P], durf[:, :], start=True, stop=True
            ).then_inc(s_mm, 1)

        @blk.vector
        def _(vector):
            n = 0  # s_v value after each op below

            def op(inst):
                nonlocal n
                n += 1
                return inst.then_inc(s_v, 1)

            def dep(k):
                vector.wait_ge(s_v, k)

            def rep_copy(j, w):
                base = j * SMAX * D
                return vector.tensor_copy(
                    out=rep[:, base + w * D : base + 2 * w * D],
                    in_=rep[:, base : base + w * D],
                )

            # --- replicated int durations + per-pass hi terms (pre-matmul)
            vector.wait_ge(s_dur, 16)
            op(vector.tensor_copy(out=dur4[:, 0:NT], in_=dur_sb[:]))  # 1
            dep(1)
            op(vector.tensor_copy(out=dur4[:, NT : 2 * NT], in_=dur4[:, 0:NT]))  # 2
            dep(2)
            op(vector.tensor_copy(out=dur4[:, 2 * NT : 4 * NT], in_=dur4[:, 0 : 2 * NT]))  # 3
            vector.wait_ge(s_ic, 16)
            dep(3)
            op(vector.tensor_tensor(
                out=hi4[:], in0=dur4[:], in1=ic_sb[:, 0:16], op=Alu.bitwise_and
            ))  # 4
            dep(3)
            op(vector.tensor_tensor(
                out=m4[:], in0=dur4[:], in1=ic_sb[:, 16:32], op=Alu.bitwise_and
            ))  # 5
            dep(5)
            op(vector.tensor_scalar(
                out=m4[:], in0=m4[:], scalar1=0, scalar2=OOB,
                op0=Alu.is_equal, op1=Alu.mult,
            ))  # 6
            dep(6)
            op(vector.tensor_tensor(out=hi4[:], in0=hi4[:], in1=m4[:], op=Alu.add))  # 7

            # --- exc, then per-pass offset groups (s=4 group first)
            vector.wait_ge(s_mm, 1)
            op(vector.tensor_copy(out=excf[:], in_=ps[:, :]))  # 8
            for sh in range(1, NT):
                dep(n)
                op(vector.tensor_tensor(
                    out=excf[:, sh:NT], in0=excf[:, sh:NT],
                    in1=cs[:, 0 : NT - sh], op=Alu.add,
                ))  # 9,10,11
            dep(11)
            op(vector.tensor_copy(out=exci[:], in_=excf[:]))  # 12 (f32->i32)
            for si in (2, 3, 1, 0):  # pass groups s=4,8,2,1
                dep(12)
                op(vector.tensor_tensor(
                    out=offs[:, si * NT : (si + 1) * NT], in0=exci[:],
                    in1=hi4[:, si * NT : (si + 1) * NT], op=Alu.add,
                ))  # 13 (s4), 14 (s8), 15 (s2), 16 (s1)

            # --- replicas for DVE_TILES
            vector.wait_ge(s_f[0], 16)
            op(rep_copy(0, 1))  # 17
            vector.wait_ge(s_f[1], 16)
            op(rep_copy(1, 1))  # 18
            dep(17)
            op(rep_copy(0, 2))  # 19
            dep(18)
            op(rep_copy(1, 2))  # 20
            dep(19)
            op(rep_copy(0, 4))  # 21
            dep(20)
            op(rep_copy(1, 4))  # 22

        @blk.gpsimd
        def _(gpsimd):
            # SWDGE indirect-ucode warm-up: 2 descriptors, both pushed past
            # bounds_check=0 -> nothing is written, but the Q7 indirect kernel
            # is hot before the real scatters
            gpsimd.memset(warm[:], OOB).then_inc(s_p, 1)
            gpsimd.memset(warmd[:], 0.0).then_inc(s_p, 1)
            gpsimd.wait_ge(s_p, 2)
            gpsimd.indirect_dma_start(
                out=out[:, :],
                out_offset=bass.IndirectOffsetOnAxis(ap=warm[0:2, 0:1], axis=0),
                in_=warmd[0:2, 0:D],
                in_offset=None,
                bounds_check=nc.gpsimd.to_reg(0),
                oob_is_err=False,
            ).then_inc(s_w, 16)
            bregs = {s_: gpsimd.to_reg(MAX_LEN - s_) for s_ in SBLK}
            for j in range(NT):
                gpsimd.wait_ge(s_f[j], 16)
            # per-scatter gates: scatter (s, tile j) waits on its own offset
            # group + tile j's replica level; DVE tiles (0,1) finish first
            v_gate = {4: {0: 19, 1: 20}, 8: {0: 21, 1: 22}, 2: {}, 1: {}}
            a_gate = {4: {2: 3, 3: 4}, 8: {2: 5, 3: 6}, 2: {2: 1, 3: 2}, 1: {}}
            v_offs = {4: 13, 8: 14, 2: 15, 1: 16}
            for s_ in ISSUE:
                si = SBLK.index(s_)
                for j in (0, 1, 2, 3):
                    gpsimd.wait_ge(s_v, max(v_offs[s_], v_gate[s_].get(j, 0)))
                    if j in a_gate[s_]:
                        gpsimd.wait_ge(s_a, a_gate[s_][j])
                    inst = gpsimd.indirect_dma_start(
                        out=out[:, :],
                        out_offset=bass.IndirectOffsetOnAxis(
                            ap=offs[:, si * NT + j : si * NT + j + 1], axis=0
                        ),
                        in_=rep[:, j * SMAX * D : j * SMAX * D + s_ * D],
                        in_offset=None,
                        bounds_check=bregs[s_],
                        oob_is_err=False,
                    ).then_inc(s_sc, 16)
                    if j:
                        inst.ins.queue = f"qPoolDynamic{j}"
            gpsimd.wait_ge(s_sc, 16 * 4 * NT)  # all 16 scatters drained
            gpsimd.wait_ge(s_w, 16)

    nc.compile()
    return nc


def _get_nc():
    if "nc" not in _CACHE:
        _CACHE["nc"] = _build_nc()
    return _CACHE["nc"]


def _run(features, durations, trace=False):
    """features (B,T,D) f32, durations (B,T) i32 -> (out (B,MAX_LEN,D) f32, results)."""
    from concourse.bass_utils import run_bass_kernel_spmd

    nc = _get_nc()
    tri, ic = _host_constants()
    in_maps = []
    for b in range(B):
        durc = np.maximum(durations[b], 1)  # clamp(min=1), as in forward()
        dmat = np.ascontiguousarray(durc.reshape(NT, P).T)  # [P, NT]
        in_maps.append(
            {
                "features": np.ascontiguousarray(features[b]),
                "durations_t": dmat,
                "durations_f": dmat.astype(np.float32),
                "tri_const": tri,
                "int_const": ic,
            }
        )
    kwargs = {}
    if trace:
        kwargs = dict(trace=True, trace_cores=list(range(B)), stitch_traces=False)
    res = run_bass_kernel_spmd(nc, in_maps, core_ids=list(range(B)), **kwargs)
    outs = np.stack([res.results[b]["out"] for b in range(B)])
    return outs.astype(np.float32, copy=False), res


def kernel(features, durations):
    features = np.asarray(features, dtype=np.float32)
    durations = np.asarray(durations, dtype=np.int32)
    outs, _ = _run(features, durations, trace=False)
    return outs


if __name__ == "__main__":
    feats = np.random.randn(B, T, D).astype(np.float32)
    durs = np.random.randint(0, 16, size=(B, T)).astype(np.int32)
    out = kernel(feats, durs)
    print("out", out.shape, out.dtype)
